# revision 54
# baseline (speedup 1.0000x reference)
# Trainium2 Bass kernel for ConvSelfAttn3D:
#   out = scale * (softmax(g @ f^T) @ h) @ Wv + x   (single head, N=4096, d=32)
#
# Sharding: 8 cores = 4 batches x 2 query-halves. Each core computes the
# full attention for its 2048 queries against all 4096 keys of its batch.
#
# Per-core layout strategy ("S-transposed flash"):
#   - All matmul operands kept in bf16 (4x faster PE than fp32), fp32 PSUM.
#   - Scores are computed transposed: S^T[key, q] via
#       matmul(lhsT=f^T tile [32,128], rhs=g^T [32, 512])
#     so softmax exp runs on [128 keys, q] tiles and the PV matmul
#       matmul(lhsT=h_aug [128,33], rhs=expS^T [128,512])
#     needs no transposes at all. h_aug has a ones column, so PV's
#     partition 32 accumulates sum_k exp(s) = the softmax denominator.
#   - Softmax max-subtraction is skipped: scores are ~N(0,1), |s| < ~6.
#   - Division by the denominator + residual happen at the very end in
#     natural layout after one PE transpose of [65, 128] tiles.
#
# Host-side prep is layout-only: transposes/casts of inputs, bias folding
# (ones rows so biases ride along in the matmuls), scale folded into Wv.

import numpy as np
import ml_dtypes

B, N, C = 4, 4096, 64
D = 32          # attn dim
NQ = N // 2     # queries per core
KT = N // 128   # 32 key tiles
NCORES = 8

_BF16 = ml_dtypes.bfloat16

# Fraction of exp tiles handled by VectorE (Schraudolph bf16 exp approx)
# instead of ScalarE ACT exp: iteration i goes to VectorE if (i % DEN) < NUM.
EXP_VEC_NUM, EXP_VEC_DEN = 1, 2
SCHRAU_A = 128.0 / float(np.log(2.0))
SCHRAU_B = 16250.5
_cache = {}


def _f32(a):
    return np.ascontiguousarray(a, dtype=np.float32)


def _bf(a):
    return np.ascontiguousarray(np.asarray(a, dtype=np.float32).astype(_BF16))


def make_shards(x, Wf, bf, Wg, bg, Wh, bh, Wv, bv, scale):
    """Host-side, layout-only sharding of the full inputs into 8 per-core maps."""
    x2 = _f32(x).reshape(B, N, C)
    ones = np.ones((1, N), np.float32)

    wf = _bf(np.concatenate([_f32(Wf), _f32(bf).reshape(1, D)], 0))        # [65,32]
    wg1 = np.concatenate([_f32(Wg), _f32(bg).reshape(1, D)], 0)
    wg = _bf(np.concatenate([wg1] * 4, 1))                                 # [65,128]
    wh_aug = np.zeros((C + 1, D + 1), np.float32)
    wh_aug[:C, :D] = _f32(Wh)
    wh_aug[C, :D] = _f32(bh)
    wh_aug[C, D] = 1.0                                                     # ones col
    wh = _bf(wh_aug)                                                       # [65,33]
    wv_aug = np.zeros((D + 1, C + 1), np.float32)
    wv_aug[:D, :C] = float(scale) * _f32(Wv)
    wv_aug[D, C] = 1.0                         # passes sumexp through v matmul
    wv = _bf(wv_aug)                                                       # [33,65]
    bvs = np.zeros((C + 1, 1), np.float32)
    bvs[:C, 0] = float(scale) * _f32(bv)                                   # [65,1]
    ident = _bf(np.eye(128, dtype=np.float32))

    in_maps = []
    for core in range(NCORES):
        b, qh = core // 2, core % 2
        xT = np.concatenate([x2[b].T, ones], 0)                            # [65,4096]
        q0 = qh * NQ
        in_maps.append({
            "xT": _bf(xT),
            "xTq": _bf(xT[:, q0:q0 + NQ]),
            "xq": _f32(x2[b, q0:q0 + NQ]),
            "wf": wf, "wg": wg, "wh": wh, "wv": wv, "bv": bvs,
            "ident": ident,
        })
    return in_maps


def build_nc():
    import concourse.bass as bass
    import concourse.mybir as mybir
    import concourse.tile as tile
    from concourse import bacc

    f32 = mybir.dt.float32
    bf16 = mybir.dt.bfloat16
    EXP = mybir.ActivationFunctionType.Exp
    MUL = mybir.AluOpType.mult
    ADD = mybir.AluOpType.add

    nc = bacc.Bacc("TRN2", target_bir_lowering=False, debug=False,
                   enable_asserts=False)

    dram = {}
    for name, shape, dt in [
        ("xT", [C + 1, N], bf16), ("xTq", [C + 1, NQ], bf16),
        ("xq", [NQ, C], f32),
        ("wf", [C + 1, D], bf16), ("wg", [C + 1, 4 * D], bf16),
        ("wh", [C + 1, D + 1], bf16), ("wv", [D + 1, C + 1], bf16),
        ("bv", [C + 1, 1], f32), ("ident", [128, 128], bf16),
    ]:
        dram[name] = nc.dram_tensor(name, shape, dt, kind="ExternalInput").ap()
    out_d = nc.dram_tensor("out", [NQ, C], f32, kind="ExternalOutput").ap()

    from contextlib import ExitStack

    with tile.TileContext(nc) as tc, ExitStack() as ctx:
        consts = ctx.enter_context(tc.tile_pool(name="consts", bufs=1))
        big = ctx.enter_context(tc.tile_pool(name="big", bufs=1))
        ps = ctx.enter_context(tc.tile_pool(name="ps", bufs=6, space="PSUM"))
        cps = ctx.enter_context(tc.tile_pool(name="cps", bufs=1, space="PSUM"))
        xs = ctx.enter_context(tc.tile_pool(name="xs", bufs=8))
        small = ctx.enter_context(tc.tile_pool(name="small", bufs=4))
        outp = ctx.enter_context(tc.tile_pool(name="outp", bufs=4))

        # ---- load constants / inputs (spread over the 3 DMA paths:
        # sync + scalar HWDGE queues, gpsimd SWDGE)
        wf_sb = consts.tile([C + 1, D], bf16)
        wg_sb = consts.tile([C + 1, 4 * D], bf16)
        wh_sb = consts.tile([C + 1, D + 1], bf16)
        wv_sb = consts.tile([D + 1, C + 1], bf16)
        bv_sb = consts.tile([C + 1, 1], f32)
        id_sb = consts.tile([128, 128], bf16)
        # prologue-critical weights go first on the HWDGE queues; epilogue-only
        # constants (wv, bv, ident) go on the slower gpsimd path.
        nc.sync.dma_start(out=wf_sb, in_=dram["wf"])
        nc.sync.dma_start(out=wh_sb, in_=dram["wh"])
        nc.scalar.dma_start(out=wg_sb, in_=dram["wg"])
        nc.gpsimd.dma_start(out=id_sb, in_=dram["ident"])
        for t, name in [(wv_sb, "wv"), (bv_sb, "bv")]:
            nc.gpsimd.dma_start(out=t, in_=dram[name])

        xT_sb = big.tile([C + 1, N], bf16)
        xTq_sb = big.tile([C + 1, NQ], bf16)
        xq_sb = big.tile([128, (NQ // 128) * C], f32)
        for c in range(4):
            sl = slice(c * 1024, (c + 1) * 1024)
            eng = nc.sync if c % 2 == 0 else nc.scalar
            eng.dma_start(out=xT_sb[:, sl], in_=dram["xT"][:, sl])
        for c in range(2):
            sl = slice(c * 1024, (c + 1) * 1024)
            eng = nc.sync if c % 2 == 0 else nc.scalar
            eng.dma_start(out=xTq_sb[:, sl], in_=dram["xTq"][:, sl])


        # ---- projections.
        # fT_pk [128, 1024]: partitions 32i+d hold f^T[d, keys of tile 4g+i]
        # at free g*128+j (kt quads packed for 4x row-tiled QK).
        # gT_rep [128, 2048]: g^T replicated on partition groups 32i.
        # h_sb [128, 32*33]: h_aug natural per key tile (col 32 = ones).
        fT_pk = big.tile([128, NQ // 2], bf16)
        gT_rep = big.tile([128, NQ], bf16)
        h_sb = big.tile([128, KT * (D + 1)], bf16)

        xT_r = xT_sb[:].rearrange("k (g i j) -> k i g j", i=4, j=128)
        for c in range(2):
            pt = ps.tile([128, 512], f32, tag="ps")
            for i in range(4):
                nc.tensor.matmul(pt[32 * i:32 * (i + 1), :], wf_sb,
                                 xT_r[:, i, 4 * c:4 * (c + 1), :],
                                 tile_position=(0, 32 * i))
            if c % 2 == 0:
                nc.scalar.copy(out=fT_pk[:, c * 512:(c + 1) * 512], in_=pt)
            else:
                nc.vector.tensor_copy(out=fT_pk[:, c * 512:(c + 1) * 512], in_=pt)

        for c in range(4):
            pt = ps.tile([128, 512], f32, tag="ps")
            nc.tensor.matmul(pt, wg_sb, xTq_sb[:, c * 512:(c + 1) * 512])
            if c % 2 == 0:
                nc.scalar.copy(out=gT_rep[:, c * 512:(c + 1) * 512], in_=pt)
            else:
                nc.vector.tensor_copy(out=gT_rep[:, c * 512:(c + 1) * 512], in_=pt)

        W1 = D + 1
        for g8 in range(KT // 4):
            pt = ps.tile([128, 4 * W1], f32, tag="ps")
            for i in range(4):
                kt = g8 * 4 + i
                nc.tensor.matmul(pt[:, i * W1:(i + 1) * W1],
                                 xT_sb[:, kt * 128:(kt + 1) * 128], wh_sb)
            if g8 % 2 == 0:
                nc.scalar.copy(out=h_sb[:, g8 * 4 * W1:(g8 + 1) * 4 * W1], in_=pt)
            else:
                nc.vector.tensor_copy(out=h_sb[:, g8 * 4 * W1:(g8 + 1) * 4 * W1], in_=pt)


        # ---- main flash loop over key-tile pairs x 512-query chunks.
        # QK: 2x row-tiled (K=32 at partitions 0/32); even kt -> s[:,0:512],
        # odd kt -> s[:,512:1024]. PV: 2x col-tiled; even kt accumulates at
        # ctx partitions 0:33, odd kt at 64:97 (merged in the epilogue).
        # Software-pipelined: PV of iteration i-1 is emitted after QK/exp of
        # iteration i, so by issue priority the PE queue is
        # [QK quad][prev PV col-pairs][next QK quad]... — tile_position
        # groups stay adjacent (running concurrently) and exp latency hides
        # under the next QK quad.
        # Two query-passes of 1024 so ctx needs only 2 PSUM banks, leaving 6
        # single-bank s-slots — enough that QK quads never stall on exp WAR.
        GH = KT // 4 - 1
        ctxA_sb = big.tile([D + 1, NQ], bf16)
        ctxT_sb = big.tile([D + 1, NQ], bf16)

        def emit_pv(prev):
            pg, plq, pex, pctx = prev[0], prev[1], prev[2], prev[3]
            for i in range(4):
                nc.tensor.matmul(
                    pctx[64 * (i % 2):64 * (i % 2) + D + 1, plq:plq + 512],
                    h_sb[:, (4 * pg + i) * W1:(4 * pg + i + 1) * W1],
                    pex[i], start=(pg == 0 and i < 2),
                    stop=(pg == GH and i >= 2),
                    skip_group_check=True)

        def emit_merge(pctx, qbase, lq):
            # merge even/odd col-group halves of one ctx chunk into ctxT_sb
            lsl = slice(lq, lq + 512)
            gsl = slice(qbase + lq, qbase + lq + 512)
            nc.scalar.copy(out=ctxA_sb[:, gsl], in_=pctx[0:D + 1, lsl])
            nc.vector.tensor_tensor(out=ctxT_sb[:, gsl],
                                    in0=ctxA_sb[:, gsl],
                                    in1=pctx[64:64 + D + 1, lsl], op=ADD)

        prev = None
        prev_ctx = None
        for qp in range(2):
            pctx = cps.tile([97, NQ // 2], f32, tag="ctx")    # 2 banks
            for g in range(KT // 4):
                for qc in range(2):
                    lq = qc * 512
                    q0 = qp * 1024 + lq
                    s = [ps.tile([128, 512], f32, tag="ps", name=f"s{i}")
                         for i in range(4)]
                    for i in range(4):
                        nc.tensor.matmul(
                            s[i],
                            fT_pk[32 * i:32 * (i + 1), g * 128:(g + 1) * 128],
                            gT_rep[32 * i:32 * (i + 1), q0:q0 + 512],
                            tile_position=(32 * i, 0))
                    ex = [xs.tile([128, 512], bf16, tag="ex", name=f"ex{i}")
                          for i in range(4)]
                    for i in range(4):
                        if i % 2 == 0:
                            a = nc.scalar.activation(out=ex[i], in_=s[i],
                                                     func=EXP)
                            if qp == 0 and g == 3 and qc == 0 and i == 0:
                                hook = a
                        else:
                            # Schraudolph bf16 exp: i16 = round(s*128/ln2 + B)
                            nc.vector.tensor_scalar(
                                out=ex[i][:].bitcast(mybir.dt.int16),
                                in0=s[i], scalar1=SCHRAU_A, scalar2=SCHRAU_B,
                                op0=MUL, op1=ADD)
                    if prev is not None:
                        emit_pv(prev)
                        if prev[0] == GH:
                            emit_merge(prev[3], prev[4] * 1024, prev[1])
                    prev = (g, lq, ex, pctx, qp)
        emit_pv(prev)
        emit_merge(prev[3], 1024, prev[1])

        # xq (residual) is only needed in the epilogue; gate its DMA on a
        # mid-loop instruction so it stays off the prologue DMA window.
        xq_dma = nc.gpsimd.dma_start(
            out=xq_sb[:].rearrange("p (t c) -> p t c", c=C),
            in_=dram["xq"].rearrange("(t p) c -> p t c", p=128),
        )
        tile.add_dep_helper(xq_dma.ins, hook.ins, sync=True,
                            reason="xq load after prologue")

        # v^T [65, q]: wv_aug row 32 / col 64 pass the sumexp row through,
        # so partition 64 of vt is the softmax denominator.
        vT_sb = big.tile([C + 1, NQ], bf16)
        for qc in range(NQ // 512):
            vt = ps.tile([C + 1, 512], f32, tag="ps")
            nc.tensor.matmul(vt, wv_sb,
                             ctxT_sb[0:D + 1, qc * 512:(qc + 1) * 512])
            if qc % 2 == 0:
                nc.vector.tensor_scalar(
                    out=vT_sb[:, qc * 512:(qc + 1) * 512], in0=vt,
                    scalar1=bv_sb, scalar2=None, op0=ADD)
            else:
                nc.scalar.activation(
                    out=vT_sb[:, qc * 512:(qc + 1) * 512], in_=vt,
                    func=mybir.ActivationFunctionType.Identity, bias=bv_sb)

        # Transpose 4 q-tiles [65, 128] into one PSUM bank [128, 4*65],
        # then do divide + residual on all 4 tiles in single strided ops.
        W2 = C + 2  # 65 written cols + 1 pad so bf16 PSUM offsets stay 4B-aligned
        for qg in range(NQ // 512):
            tp = ps.tile([128, 4 * W2], bf16, tag="ps")
            for t in range(4):
                qt = qg * 4 + t
                nc.tensor.matmul(tp[:, t * W2:t * W2 + C + 1],
                                 vT_sb[:, qt * 128:(qt + 1) * 128],
                                 id_sb[0:C + 1, 0:C + 1], is_transpose=True,
                                 skip_group_check=True)
            tp3 = tp[:].rearrange("p (t c) -> p t c", c=W2)
            r = small.tile([128, 4, 1], f32, tag="r")
            nc.vector.reciprocal(r, tp3[:, :, C:C + 1])
            rap = r[:]
            rb = bass.AP(tensor=rap.tensor, offset=rap.offset,
                         ap=[list(rap.ap[0]), list(rap.ap[1]), [0, C]])
            tmp = outp.tile([128, 4, C], f32, tag="tmp")
            nc.vector.tensor_tensor(out=tmp, in0=tp3[:, :, 0:C], in1=rb,
                                    op=MUL)
            ot = outp.tile([128, 4, C], f32, tag="ot")
            nc.gpsimd.tensor_tensor(
                out=ot, in0=tmp,
                in1=xq_sb[:].rearrange("p (t c) -> p t c", c=C)[:, 4 * qg:4 * qg + 4, :],
                op=ADD)
            nc.sync.dma_start(
                out=out_d.rearrange("(t p) c -> p t c", p=128)[:, 4 * qg:4 * qg + 4, :],
                in_=ot)

    nc.compile()
    return nc


def get_nc():
    if "nc" not in _cache:
        _cache["nc"] = build_nc()
    return _cache["nc"]


def kernel(**inputs):
    from concourse.bass_utils import run_bass_kernel_spmd

    nc = get_nc()
    in_maps = make_shards(**inputs)
    res = run_bass_kernel_spmd(nc, in_maps, core_ids=list(range(NCORES)))
    out = np.empty((B, N, C), np.float32)
    for core in range(NCORES):
        b, qh = core // 2, core % 2
        out[b, qh * NQ:(qh + 1) * NQ] = res.results[core]["out"]
    return out.reshape(B, 16, 16, 16, C)


# revision 55
# speedup vs baseline: 1.1851x; 1.1851x over previous
# Trainium2 Bass kernel for ConvSelfAttn3D:
#   out = scale * (softmax(g @ f^T) @ h) @ Wv + x   (single head, N=4096, d=32)
#
# Sharding: 8 cores = 4 batches x 2 query-halves. Each core computes the
# full attention for its 2048 queries against all 4096 keys of its batch.
#
# Per-core layout strategy ("S-transposed flash"):
#   - All matmul operands kept in bf16 (4x faster PE than fp32), fp32 PSUM.
#   - Scores are computed transposed: S^T[key, q] via
#       matmul(lhsT=f^T tile [32,128], rhs=g^T [32, 512])
#     so softmax exp runs on [128 keys, q] tiles and the PV matmul
#       matmul(lhsT=h_aug [128,33], rhs=expS^T [128,512])
#     needs no transposes at all. h_aug has a ones column, so PV's
#     partition 32 accumulates sum_k exp(s) = the softmax denominator.
#   - Softmax max-subtraction is skipped: scores are ~N(0,1), |s| < ~6.
#   - Division by the denominator + residual happen at the very end in
#     natural layout after one PE transpose of [65, 128] tiles.
#
# Host-side prep is layout-only: transposes/casts of inputs, bias folding
# (ones rows so biases ride along in the matmuls), scale folded into Wv.

import numpy as np
import ml_dtypes

B, N, C = 4, 4096, 64
D = 32          # attn dim
NQ = N // 2     # queries per core
KT = N // 128   # 32 key tiles
NCORES = 8

_BF16 = ml_dtypes.bfloat16

# Fraction of exp tiles handled by VectorE (Schraudolph bf16 exp approx)
# instead of ScalarE ACT exp: iteration i goes to VectorE if (i % DEN) < NUM.
EXP_VEC_NUM, EXP_VEC_DEN = 1, 2
SCHRAU_A = 128.0 / float(np.log(2.0))
SCHRAU_B = 16250.5
_cache = {}


def _f32(a):
    return np.ascontiguousarray(a, dtype=np.float32)


def _bf(a):
    return np.ascontiguousarray(np.asarray(a, dtype=np.float32).astype(_BF16))


def make_shards(x, Wf, bf, Wg, bg, Wh, bh, Wv, bv, scale):
    """Host-side, layout-only sharding of the full inputs into 8 per-core maps."""
    x2 = _f32(x).reshape(B, N, C)
    ones = np.ones((1, N), np.float32)

    wf = _bf(np.concatenate([_f32(Wf), _f32(bf).reshape(1, D)], 0))        # [65,32]
    wg1 = np.concatenate([_f32(Wg), _f32(bg).reshape(1, D)], 0)
    wg = _bf(np.concatenate([wg1] * 4, 1))                                 # [65,128]
    wh_aug = np.zeros((C + 1, D + 1), np.float32)
    wh_aug[:C, :D] = _f32(Wh)
    wh_aug[C, :D] = _f32(bh)
    wh_aug[C, D] = 1.0                                                     # ones col
    wh = _bf(wh_aug)                                                       # [65,33]
    wv_aug = np.zeros((D + 1, C + 1), np.float32)
    wv_aug[:D, :C] = float(scale) * _f32(Wv)
    wv_aug[D, C] = 1.0                         # passes sumexp through v matmul
    wv = _bf(wv_aug)                                                       # [33,65]
    bvs = np.zeros((C + 1, 1), np.float32)
    bvs[:C, 0] = float(scale) * _f32(bv)                                   # [65,1]
    ident = _bf(np.eye(128, dtype=np.float32))

    in_maps = []
    for core in range(NCORES):
        b, qh = core // 2, core % 2
        xT = np.concatenate([x2[b].T, ones], 0)                            # [65,4096]
        q0 = qh * NQ
        in_maps.append({
            "xT": _bf(xT),
            "xTq": _bf(xT[:, q0:q0 + NQ]),
            "xq": _f32(x2[b, q0:q0 + NQ]),
            "wf": wf, "wg": wg, "wh": wh, "wv": wv, "bv": bvs,
            "ident": ident,
        })
    return in_maps


def build_nc():
    import concourse.bass as bass
    import concourse.mybir as mybir
    import concourse.tile as tile
    from concourse import bacc

    f32 = mybir.dt.float32
    bf16 = mybir.dt.bfloat16
    EXP = mybir.ActivationFunctionType.Exp
    MUL = mybir.AluOpType.mult
    ADD = mybir.AluOpType.add

    nc = bacc.Bacc("TRN2", target_bir_lowering=False, debug=False,
                   enable_asserts=False)

    dram = {}
    for name, shape, dt in [
        ("xT", [C + 1, N], bf16), ("xTq", [C + 1, NQ], bf16),
        ("xq", [NQ, C], f32),
        ("wf", [C + 1, D], bf16), ("wg", [C + 1, 4 * D], bf16),
        ("wh", [C + 1, D + 1], bf16), ("wv", [D + 1, C + 1], bf16),
        ("bv", [C + 1, 1], f32), ("ident", [128, 128], bf16),
    ]:
        dram[name] = nc.dram_tensor(name, shape, dt, kind="ExternalInput").ap()
    out_d = nc.dram_tensor("out", [NQ, C], f32, kind="ExternalOutput").ap()

    from contextlib import ExitStack

    with tile.TileContext(nc) as tc, ExitStack() as ctx:
        consts = ctx.enter_context(tc.tile_pool(name="consts", bufs=1))
        big = ctx.enter_context(tc.tile_pool(name="big", bufs=1))
        ps = ctx.enter_context(tc.tile_pool(name="ps", bufs=6, space="PSUM"))
        cps = ctx.enter_context(tc.tile_pool(name="cps", bufs=1, space="PSUM"))
        xs = ctx.enter_context(tc.tile_pool(name="xs", bufs=8))
        small = ctx.enter_context(tc.tile_pool(name="small", bufs=4))
        outp = ctx.enter_context(tc.tile_pool(name="outp", bufs=4))

        # ---- load constants / inputs (spread over the 3 DMA paths:
        # sync + scalar HWDGE queues, gpsimd SWDGE)
        wf_sb = consts.tile([C + 1, D], bf16)
        wg_sb = consts.tile([C + 1, 4 * D], bf16)
        wh_sb = consts.tile([C + 1, D + 1], bf16)
        wv_sb = consts.tile([D + 1, C + 1], bf16)
        bv_sb = consts.tile([C + 1, 1], f32)
        id_sb = consts.tile([128, 128], bf16)
        # prologue-critical weights go first on the HWDGE queues; epilogue-only
        # constants (wv, bv, ident) go on the slower gpsimd path.
        nc.sync.dma_start(out=wf_sb, in_=dram["wf"])
        nc.sync.dma_start(out=wh_sb, in_=dram["wh"])
        nc.scalar.dma_start(out=wg_sb, in_=dram["wg"])
        nc.gpsimd.dma_start(out=id_sb, in_=dram["ident"])
        for t, name in [(wv_sb, "wv"), (bv_sb, "bv")]:
            nc.gpsimd.dma_start(out=t, in_=dram[name])

        xT_sb = big.tile([C + 1, N], bf16)
        xTq_sb = big.tile([C + 1, NQ], bf16)
        xq_sb = big.tile([128, (NQ // 128) * C], f32)
        for c in range(4):
            sl = slice(c * 1024, (c + 1) * 1024)
            eng = nc.sync if c % 2 == 0 else nc.scalar
            eng.dma_start(out=xT_sb[:, sl], in_=dram["xT"][:, sl])
        for c in range(2):
            sl = slice(c * 1024, (c + 1) * 1024)
            eng = nc.sync if c % 2 == 0 else nc.scalar
            eng.dma_start(out=xTq_sb[:, sl], in_=dram["xTq"][:, sl])


        # ---- projections.
        # fT_pk [128, 1024]: partitions 32i+d hold f^T[d, keys of tile 4g+i]
        # at free g*128+j (kt quads packed for 4x row-tiled QK).
        # gT_rep [128, 2048]: g^T replicated on partition groups 32i.
        # h_sb [128, 32*33]: h_aug natural per key tile (col 32 = ones).
        fT_pk = big.tile([128, NQ // 2], bf16)
        gT_rep = big.tile([128, NQ], bf16)
        h_sb = big.tile([128, KT * (D + 1)], bf16)

        xT_r = xT_sb[:].rearrange("k (g i j) -> k i g j", i=4, j=128)
        for c in range(2):
            pt = ps.tile([128, 512], f32, tag="ps")
            for i in range(4):
                nc.tensor.matmul(pt[32 * i:32 * (i + 1), :], wf_sb,
                                 xT_r[:, i, 4 * c:4 * (c + 1), :],
                                 tile_position=(0, 32 * i))
            if c % 2 == 0:
                nc.scalar.copy(out=fT_pk[:, c * 512:(c + 1) * 512], in_=pt)
            else:
                nc.vector.tensor_copy(out=fT_pk[:, c * 512:(c + 1) * 512], in_=pt)

        for c in range(4):
            pt = ps.tile([128, 512], f32, tag="ps")
            nc.tensor.matmul(pt, wg_sb, xTq_sb[:, c * 512:(c + 1) * 512])
            if c % 2 == 0:
                nc.scalar.copy(out=gT_rep[:, c * 512:(c + 1) * 512], in_=pt)
            else:
                nc.vector.tensor_copy(out=gT_rep[:, c * 512:(c + 1) * 512], in_=pt)

        W1 = D + 1
        for g8 in range(KT // 4):
            pt = ps.tile([128, 4 * W1], f32, tag="ps")
            for i in range(4):
                kt = g8 * 4 + i
                nc.tensor.matmul(pt[:, i * W1:(i + 1) * W1],
                                 xT_sb[:, kt * 128:(kt + 1) * 128], wh_sb)
            if g8 % 2 == 0:
                nc.scalar.copy(out=h_sb[:, g8 * 4 * W1:(g8 + 1) * 4 * W1], in_=pt)
            else:
                nc.vector.tensor_copy(out=h_sb[:, g8 * 4 * W1:(g8 + 1) * 4 * W1], in_=pt)


        # ---- main flash loop over key-tile pairs x 512-query chunks.
        # QK: 2x row-tiled (K=32 at partitions 0/32); even kt -> s[:,0:512],
        # odd kt -> s[:,512:1024]. PV: 2x col-tiled; even kt accumulates at
        # ctx partitions 0:33, odd kt at 64:97 (merged in the epilogue).
        # Software-pipelined: PV of iteration i-1 is emitted after QK/exp of
        # iteration i, so by issue priority the PE queue is
        # [QK quad][prev PV col-pairs][next QK quad]... — tile_position
        # groups stay adjacent (running concurrently) and exp latency hides
        # under the next QK quad.
        # Two query-passes of 1024 so ctx needs only 2 PSUM banks, leaving 6
        # single-bank s-slots — enough that QK quads never stall on exp WAR.
        GH = KT // 4 - 1
        ctxA_sb = big.tile([D + 1, NQ], bf16)
        ctxT_sb = big.tile([D + 1, NQ], bf16)

        def emit_pv(prev):
            pg, plq, pex, pctx = prev[0], prev[1], prev[2], prev[3]
            for i in range(4):
                nc.tensor.matmul(
                    pctx[64 * (i % 2):64 * (i % 2) + D + 1, plq:plq + 512],
                    h_sb[:, (4 * pg + i) * W1:(4 * pg + i + 1) * W1],
                    pex[i], start=(pg == 0 and i < 2),
                    stop=(pg == GH and i >= 2),
                    skip_group_check=True)

        def emit_merge(pctx, qbase, lq):
            # merge even/odd col-group halves of one ctx chunk into ctxT_sb
            lsl = slice(lq, lq + 512)
            gsl = slice(qbase + lq, qbase + lq + 512)
            nc.scalar.copy(out=ctxA_sb[:, gsl], in_=pctx[0:D + 1, lsl])
            nc.vector.tensor_tensor(out=ctxT_sb[:, gsl],
                                    in0=ctxA_sb[:, gsl],
                                    in1=pctx[64:64 + D + 1, lsl], op=ADD)

        prev = None
        prev_ctx = None
        for qp in range(2):
            pctx = cps.tile([97, NQ // 2], f32, tag="ctx")    # 2 banks
            for g in range(KT // 4):
                for qc in range(2):
                    lq = qc * 512
                    q0 = qp * 1024 + lq
                    s = [ps.tile([128, 512], f32, tag="ps", name=f"s{i}")
                         for i in range(4)]
                    for i in range(4):
                        nc.tensor.matmul(
                            s[i],
                            fT_pk[32 * i:32 * (i + 1), g * 128:(g + 1) * 128],
                            gT_rep[32 * i:32 * (i + 1), q0:q0 + 512],
                            tile_position=(32 * i, 0))
                    ex = [xs.tile([128, 512], bf16, tag="ex", name=f"ex{i}")
                          for i in range(4)]
                    for i in range(4):
                        if i % 2 == 0:
                            a = nc.scalar.activation(out=ex[i], in_=s[i],
                                                     func=EXP)
                            if qp == 0 and g == 3 and qc == 0 and i == 0:
                                hook = a
                        else:
                            # Schraudolph bf16 exp: i16 = round(s*128/ln2 + B)
                            nc.vector.tensor_scalar(
                                out=ex[i][:].bitcast(mybir.dt.int16),
                                in0=s[i], scalar1=SCHRAU_A, scalar2=SCHRAU_B,
                                op0=MUL, op1=ADD)
                    if prev is not None:
                        emit_pv(prev)
                        if prev[0] == GH and prev[1] == 512:
                            emit_merge(prev[3], prev[4] * 1024, 0)
                            emit_merge(prev[3], prev[4] * 1024, 512)
                    prev = (g, lq, ex, pctx, qp)
        emit_pv(prev)
        emit_merge(prev[3], 1024, 0)
        emit_merge(prev[3], 1024, 512)

        # xq (residual) is only needed in the epilogue; gate its DMA on a
        # mid-loop instruction so it stays off the prologue DMA window.
        xq_dma = nc.gpsimd.dma_start(
            out=xq_sb[:].rearrange("p (t c) -> p t c", c=C),
            in_=dram["xq"].rearrange("(t p) c -> p t c", p=128),
        )
        tile.add_dep_helper(xq_dma.ins, hook.ins, sync=True,
                            reason="xq load after prologue")

        # v^T [65, q]: wv_aug row 32 / col 64 pass the sumexp row through,
        # so partition 64 of vt is the softmax denominator.
        vT_sb = big.tile([C + 1, NQ], bf16)
        for qc in range(NQ // 512):
            vt = ps.tile([C + 1, 512], f32, tag="ps")
            nc.tensor.matmul(vt, wv_sb,
                             ctxT_sb[0:D + 1, qc * 512:(qc + 1) * 512])
            if qc % 2 == 0:
                nc.vector.tensor_scalar(
                    out=vT_sb[:, qc * 512:(qc + 1) * 512], in0=vt,
                    scalar1=bv_sb, scalar2=None, op0=ADD)
            else:
                nc.scalar.activation(
                    out=vT_sb[:, qc * 512:(qc + 1) * 512], in_=vt,
                    func=mybir.ActivationFunctionType.Identity, bias=bv_sb)

        # Transpose 4 q-tiles [65, 128] into one PSUM bank [128, 4*65],
        # then do divide + residual on all 4 tiles in single strided ops.
        W2 = C + 2  # 65 written cols + 1 pad so bf16 PSUM offsets stay 4B-aligned
        for qg in range(NQ // 512):
            tp = ps.tile([128, 4 * W2], bf16, tag="ps")
            for t in range(4):
                qt = qg * 4 + t
                nc.tensor.matmul(tp[:, t * W2:t * W2 + C + 1],
                                 vT_sb[:, qt * 128:(qt + 1) * 128],
                                 id_sb[0:C + 1, 0:C + 1], is_transpose=True,
                                 skip_group_check=True)
            tp3 = tp[:].rearrange("p (t c) -> p t c", c=W2)
            r = small.tile([128, 4, 1], f32, tag="r")
            nc.vector.reciprocal(r, tp3[:, :, C:C + 1])
            rap = r[:]
            rb = bass.AP(tensor=rap.tensor, offset=rap.offset,
                         ap=[list(rap.ap[0]), list(rap.ap[1]), [0, C]])
            tmp = outp.tile([128, 4, C], f32, tag="tmp")
            nc.vector.tensor_tensor(out=tmp, in0=tp3[:, :, 0:C], in1=rb,
                                    op=MUL)
            ot = outp.tile([128, 4, C], f32, tag="ot")
            nc.gpsimd.tensor_tensor(
                out=ot, in0=tmp,
                in1=xq_sb[:].rearrange("p (t c) -> p t c", c=C)[:, 4 * qg:4 * qg + 4, :],
                op=ADD)
            nc.sync.dma_start(
                out=out_d.rearrange("(t p) c -> p t c", p=128)[:, 4 * qg:4 * qg + 4, :],
                in_=ot)

    nc.compile()
    return nc


def get_nc():
    if "nc" not in _cache:
        _cache["nc"] = build_nc()
    return _cache["nc"]


def kernel(**inputs):
    from concourse.bass_utils import run_bass_kernel_spmd

    nc = get_nc()
    in_maps = make_shards(**inputs)
    res = run_bass_kernel_spmd(nc, in_maps, core_ids=list(range(NCORES)))
    out = np.empty((B, N, C), np.float32)
    for core in range(NCORES):
        b, qh = core // 2, core % 2
        out[b, qh * NQ:(qh + 1) * NQ] = res.results[core]["out"]
    return out.reshape(B, 16, 16, 16, C)
